# revision 1
# baseline (speedup 1.0000x reference)
"""Trainium2 Bass kernel for the AOI (attention-over-interactions) module.

Strategy (8 NeuronCores, data-parallel over question groups):
  - B=128 options = 32 self-contained groups of 4; 4 "group slots" x 8 cores.
  - Host assigns groups to slots to minimize per-slot ragged length maxima
    (same SPMD graph on all cores; per-(slot,option) key lengths baked into
    the instruction stream as max over the 8 cores at that slot).
  - Per-core masked-softmax semantics are matched to the reference exactly
    (max over masked-zeros, i.e. M = max(0, row max), and the +1e-13
    renormalization epsilon scaled by the full-softmax partition function)
    via a per-core additive mask (host-broadcast to 128 partitions, applied
    with one vector add per score tile) plus one extra "epsilon column".
  - Mixed precision: bf16 for the attention-score path, fp32 accumulation
    everywhere.
"""

import math
import sys

for _p in ("/opt/trn_rl_repo", "/opt/pypackages"):
    if _p not in sys.path:
        sys.path.append(_p)

import numpy as np
import ml_dtypes

B, S, H = 128, 256, 768
N_CORES = 8
GPC = 4           # group slots per core
HC = H // 128     # 6 h-chunks
BF16 = ml_dtypes.bfloat16
FP8 = ml_dtypes.float8_e4m3   # TRN float8e4; max 240, overflow -> inf
SCW = 2048.0      # fp8 scale for Wd3
SCK = 64.0        # fp8 scale for ock


def _clip8(x, scale):
    return np.clip(np.asarray(x, np.float32) * scale, -224.0, 224.0).astype(FP8)

_GRAPH_CACHE = {}


def _assign_groups(glens):
    """Partition 32 groups into 4 slots of 8 minimizing sum_slot sum_o max_core len.

    Returns slots: list[4] of list[8] group ids (core c takes slots[g][c]).
    """
    rng = np.random.default_rng(0)
    n_groups = glens.shape[0]

    def cost(assign):
        c = 0
        for g in range(GPC):
            ids = assign[g]
            c += int(glens[ids].max(axis=0).sum())
        return c

    best, best_cost = None, None
    for trial in range(6):
        if trial == 0:
            order = np.argsort(-glens.sum(axis=1))
        else:
            order = rng.permutation(n_groups)
        assign = [list(order[g * 8:(g + 1) * 8]) for g in range(GPC)]
        # 2-swap hill climbing across slots
        improved = True
        cur = cost(assign)
        it = 0
        while improved and it < 60:
            improved = False
            it += 1
            for ga in range(GPC):
                for gb in range(ga + 1, GPC):
                    for ia in range(8):
                        for ib in range(8):
                            assign[ga][ia], assign[gb][ib] = assign[gb][ib], assign[ga][ia]
                            nc_ = cost(assign)
                            if nc_ < cur:
                                cur = nc_
                                improved = True
                            else:
                                assign[ga][ia], assign[gb][ib] = assign[gb][ib], assign[ga][ia]
        if best_cost is None or cur < best_cost:
            best_cost, best = cur, [list(a) for a in assign]
    return best


def _build_graph(slot_lens):
    """Build + compile the SPMD Bacc graph. slot_lens: [GPC][4] ints (1..256)."""
    import concourse.bass as bass
    import concourse.bacc as bacc
    import concourse.mybir as mybir
    import concourse.tile as tile
    from concourse.masks import make_identity

    f32 = mybir.dt.float32
    bf = mybir.dt.bfloat16
    f8 = mybir.dt.float8e4
    AX = mybir.AxisListType
    AF = mybir.ActivationFunctionType
    DR = mybir.MatmulPerfMode.DoubleRow

    nc = bacc.Bacc("TRN2", target_bir_lowering=False, debug=False,
                   num_devices=N_CORES)

    p_bf_d = nc.dram_tensor("p_bf", [GPC, HC, 128, 4, S], bf, kind="ExternalInput")
    mask_d = nc.dram_tensor("maskbc", [GPC, 128, 4, S + 1], bf, kind="ExternalInput")
    wt_d = nc.dram_tensor("wt", [128, HC, HC, 128], bf, kind="ExternalInput")
    wd_d = nc.dram_tensor("wd", [128, HC, H], bf, kind="ExternalInput")
    wd3_d = nc.dram_tensor("wd3", [3, 128, 3, HC, 2, 128], f8, kind="ExternalInput")
    w1_d = nc.dram_tensor("w1", [128, HC, HC, 128], bf, kind="ExternalInput")
    w2_d = nc.dram_tensor("w2", [128, HC, HC, 128], bf, kind="ExternalInput")
    bias_d = nc.dram_tensor("biases", [128, 4, HC], f32, kind="ExternalInput")
    out_d = nc.dram_tensor("out", [GPC, 4, HC, 128, S], f32, kind="ExternalOutput")

    with tile.TileContext(nc) as tc:
        with (
            tc.tile_pool(name="const", bufs=1) as constp,
            tc.tile_pool(name="wres", bufs=1) as wres,
            tc.tile_pool(name="wstream", bufs=2) as wstream,
            tc.tile_pool(name="gin", bufs=1) as gin,
            tc.tile_pool(name="act", bufs=1) as act,
            tc.tile_pool(name="smp", bufs=2) as smp,
            tc.tile_pool(name="fin", bufs=2) as fin,
            tc.tile_pool(name="patt", bufs=4, space="PSUM") as patt,
            tc.tile_pool(name="psbig", bufs=1, space="PSUM") as psbig,
            tc.tile_pool(name="pshalf", bufs=1, space="PSUM") as pshalf,
            tc.tile_pool(name="pstr", bufs=1, space="PSUM") as pstr,
        ):
            ident_f = constp.tile([128, 128], f32)
            make_identity(nc, ident_f[:])
            ident = constp.tile([128, 128], bf)
            nc.vector.tensor_copy(ident[:], ident_f[:])
            biases = constp.tile([128, 4, HC], f32)
            nc.sync.dma_start(biases[:], bias_d.ap())
            bt_b = biases[:, 0, :]
            bd_b = biases[:, 1, :]
            bd3_b = biases[:, 2, :]
            b12_b = biases[:, 3, :]

            wt_sb = wres.tile([128, HC, HC, 128], bf)
            nc.sync.dma_start(wt_sb[:], wt_d.ap())

            # first group's inputs land before the remaining weights so the
            # tensor engine can start early
            pbf_t, msk_t = [], []
            for g in range(GPC):
                pbf_t.append(gin.tile([128, HC, 4, S], bf, tag="pbf", bufs=2,
                                      name=f"pbf{g}"))
                msk_t.append(gin.tile([128, 4, S + 1], bf, tag="msk", bufs=2,
                                      name=f"msk{g}"))

            def load_group(g):
                for hc in range(HC):
                    nc.sync.dma_start(pbf_t[g][:, hc, :, :], p_bf_d.ap()[g][hc])
                nc.sync.dma_start(msk_t[g][:], mask_d.ap()[g])

            load_group(0)

            wd_sb = wres.tile([128, HC, H], bf)
            nc.sync.dma_start(wd_sb[:], wd_d.ap())
            w1_sb = wres.tile([128, HC, HC, 128], bf)
            nc.sync.dma_start(w1_sb[:], w1_d.ap())
            w2_sb = wres.tile([128, HC, HC, 128], bf)
            nc.sync.dma_start(w2_sb[:], w2_d.ap())

            for g in range(GPC):
                L = [int(x) for x in slot_lens[g]]
                tcs = [max(1, math.ceil(l / 128)) for l in L]

                pbf, mskb = pbf_t[g], msk_t[g]
                if g + 1 < GPC:
                    load_group(g + 1)

                # ---- trans_t (feature-major, bf16): ttT[h', t] = Wt @ pT + bt
                # one extra column per option (index L[o]) zeroed so the score
                # matmuls can produce the epsilon column via accumulation
                ttT = act.tile([128, HC, 4, S + 1], bf, tag="ttT", bufs=2)
                for m in range(HC):
                    for o in range(4):
                        ps = patt.tile([128, 257], f32, tag="patt")
                        for hc in range(HC):
                            nc.tensor.matmul(
                                ps[:, 0:L[o]],
                                wt_sb[:, hc, m, :],
                                pbf[:, hc, o, 0:L[o]],
                                start=(hc == 0), stop=(hc == HC - 1),
                            )
                        nc.scalar.activation(
                            ttT[:, m, o, 0:L[o]], ps[:, 0:L[o]],
                            AF.Identity, bias=bt_b[:, m:m + 1],
                        )
                for o in range(4):
                    nc.vector.memset(ttT[:, :, o, L[o]:L[o] + 1], 0.0)

                # ---- trans_d (natural [t, h], bf16): td = pT^T @ WdT
                # td psum deliberately avoids the psbig tag: sharing it with
                # the final stage's zps (bufs=1) stalled td(g+1) on the last
                # sigmoid of group g at every group boundary
                td = act.tile([128, 4, 2, H], bf, tag="td")
                for o in range(4):
                    for tcx in range(tcs[o]):
                        w = min(128, L[o] - tcx * 128)
                        psA = pshalf.tile([128, 512], f32, tag="pshalf",
                                          name="tdA")
                        psB = patt.tile([128, 257], f32, tag="patt", name="tdB")
                        for hc in range(HC):
                            lhs = pbf[:, hc, o, tcx * 128: tcx * 128 + w]
                            nc.tensor.matmul(psA[0:w, 0:512], lhs, wd_sb[:, hc, 0:512],
                                             start=(hc == 0), stop=(hc == HC - 1))
                            nc.tensor.matmul(psB[0:w, 0:256], lhs, wd_sb[:, hc, 512:768],
                                             start=(hc == 0), stop=(hc == HC - 1))
                        nc.scalar.activation(td[0:w, o, tcx, 0:512], psA[0:w, 0:512],
                                             AF.Copy)
                        nc.scalar.activation(td[0:w, o, tcx, 512:768], psB[0:w, 0:256],
                                             AF.Copy)

                # stream Wd3 for this group
                wd3_sb = []
                for k in range(3):
                    t = wstream.tile([128, 3, HC, 2, 128], f8, tag=f"wd3_{k}", bufs=1)
                    nc.sync.dma_start(t[:], wd3_d.ap()[k])
                    wd3_sb.append(t)

                ocT = act.tile([128, HC, 4, S], bf, tag="ocT")
                for half in range(2):
                    ock = act.tile([128, 3, HC, 2, S], f8, tag="ock", bufs=2)
                    for io in range(2):
                        i = half * 2 + io
                        jlist = [j for j in range(4) if j != i]

                        def emit_scores(j):
                            # scores + softmax chain for one partner; the
                            # vector/scalar chain overlaps the next partner's
                            # score matmuls (software pipelining)
                            lj = L[j]
                            smc = []
                            for m in range(2):
                                ps = patt.tile([128, 257], f32, tag="patt",
                                               name="ps")
                                for hc in range(HC):
                                    nc.tensor.matmul(
                                        ps[:, 0:lj + 1],
                                        pbf[:, hc, i, m * 128:(m + 1) * 128],
                                        ttT[:, hc, j, 0:lj + 1],
                                        start=(hc == 0), stop=(hc == HC - 1),
                                    )
                                nc.vector.tensor_add(
                                    ps[:, 0:lj + 1], ps[:, 0:lj + 1],
                                    mskb[:, j, 0:lj + 1])
                                stats = smp.tile([128, 16], f32, tag="stats",
                                                 bufs=8, name="stats")
                                nc.vector.tensor_reduce(
                                    stats[:, 0:1], ps[:, 0:lj], AX.X,
                                    mybir.AluOpType.max, negate=True)
                                # -M = min(0, -max)
                                nc.vector.tensor_scalar_min(stats[:, 0:1], stats[:, 0:1], 0.0)
                                e = smp.tile([128, 257], f32, tag="e", bufs=4,
                                             name="e")
                                nc.scalar.activation(
                                    e[:, 0:lj + 1], ps[:, 0:lj + 1],
                                    AF.Exp, bias=stats[:, 0:1], scale=1.0,
                                    accum_out=stats[:, 1:2])
                                nc.vector.reciprocal(stats[:, 2:3], stats[:, 1:2])
                                sm = smp.tile([128, 256], bf, tag="sm", bufs=6,
                                              name="sm")
                                nc.vector.tensor_scalar_mul(sm[:, 0:lj], e[:, 0:lj],
                                                            stats[:, 2:3])
                                smc.append(sm)
                            return smc

                        def emit_transpose(j, smc):
                            lj = L[j]
                            smT = smp.tile([128, 2, 256], bf, tag="smT", bufs=3,
                                           name="smT")
                            for tcx in range(tcs[j]):
                                w = min(128, lj - tcx * 128)
                                tp = pstr.tile([128, 256], bf, tag="pstr",
                                               name="tp")
                                for m in range(2):
                                    nc.tensor.matmul(
                                        tp[0:w, m * 128:(m + 1) * 128],
                                        smc[m][:, tcx * 128: tcx * 128 + w],
                                        ident[:], is_transpose=True,
                                        start=(m == 0), stop=(m == 1))
                                nc.scalar.activation(smT[0:w, tcx, :], tp[0:w, :],
                                                     AF.Copy)
                            return smT

                        def emit_av(j, jr, smT):
                            # av: avT[h', s] += td_j^T(blocks) @ smT
                            lj = L[j]
                            for hc in range(HC):
                                aps = patt.tile([128, 257], f32, tag="patt",
                                                name="aps")
                                for tcx in range(tcs[j]):
                                    w = min(128, lj - tcx * 128)
                                    nc.tensor.matmul(
                                        aps[:, 0:256],
                                        td[0:w, j, tcx, hc * 128:(hc + 1) * 128],
                                        smT[0:w, tcx, :],
                                        start=(tcx == 0), stop=(tcx == tcs[j] - 1))
                                nc.scalar.activation(
                                    ock[:, jr, hc, io, :], aps[:, 0:256],
                                    AF.Relu, bias=bd_b[:, hc:hc + 1], scale=SCK)

                        smc0 = emit_scores(jlist[0])
                        smc1 = emit_scores(jlist[1])
                        smT0 = emit_transpose(jlist[0], smc0)
                        emit_av(jlist[0], 0, smT0)
                        smc2 = emit_scores(jlist[2])
                        smT1 = emit_transpose(jlist[1], smc1)
                        emit_av(jlist[1], 1, smT1)
                        smT2 = emit_transpose(jlist[2], smc2)
                        emit_av(jlist[2], 2, smT2)

                    # ---- oc for this half (2 options): ocT = sum_k Wd3_k @ ock_k + bd3
                    for m in range(HC):
                        ops = pshalf.tile([128, 512], f32, tag="pshalf")
                        step = 0
                        for k in range(3):
                            for hp in range(3):
                                nc.tensor.matmul(
                                    ops[:, :], wd3_sb[k][:, hp, m, :, :],
                                    ock[:, k, 2 * hp:2 * hp + 2, :, :],
                                    start=(step == 0), stop=(step == 8),
                                    perf_mode=DR)
                                step += 1
                        for oo in range(2):
                            o = half * 2 + oo
                            nc.scalar.activation(
                                ocT[:, m, o, :], ops[:, oo * 256:(oo + 1) * 256],
                                AF.Identity, bias=bd3_b[:, m:m + 1],
                                scale=1.0 / (SCK * SCW))

                # ---- mid + final output, per m-chunk
                for m in range(HC):
                    # w2 @ p first: no dependency on the oc stage, so the PE
                    # rolls straight into the final stage while the last ocT
                    # activations drain
                    zps = psbig.tile([128, 1024], f32, tag="psbig")
                    for hc in range(HC):
                        nc.tensor.matmul(zps[:, 0:512], w2_sb[:, hc, m, :],
                                         pbf[:, hc, 0:2, :],
                                         start=(hc == 0), stop=False)
                        nc.tensor.matmul(zps[:, 512:1024], w2_sb[:, hc, m, :],
                                         pbf[:, hc, 2:4, :],
                                         start=(hc == 0), stop=False)
                    for hc in range(HC):
                        nc.tensor.matmul(zps[:, 0:512], w1_sb[:, hc, m, :],
                                         ocT[:, hc, 0:2, :],
                                         start=False, stop=(hc == HC - 1))
                        nc.tensor.matmul(zps[:, 512:1024], w1_sb[:, hc, m, :],
                                         ocT[:, hc, 2:4, :],
                                         start=False, stop=(hc == HC - 1))
                    mid = fin.tile([128, 1024], bf, tag="mid", bufs=2)
                    nc.scalar.activation(mid[:], zps[:], AF.Sigmoid,
                                         bias=b12_b[:, m:m + 1])
                    for o in range(4):
                        d = fin.tile([128, 256], f32, tag="fd", bufs=3)
                        nc.gpsimd.tensor_sub(d[:], pbf[:, m, o, :], ocT[:, m, o, :])
                        nc.vector.tensor_mul(d[:], d[:], mid[:, o * 256:(o + 1) * 256])
                        fo = fin.tile([128, 256], f32, tag="fout", bufs=4)
                        nc.vector.tensor_add(fo[:], d[:], ocT[:, m, o, :])
                        nc.sync.dma_start(out_d.ap()[g][o][m], fo[:])

    nc.compile()
    return nc


def _pack_weights(Wt, bt, Wd, bd, Wd3, bd3, W1, b1, W2, b2):
    def lhs_blocks(w):  # [H,H] -> [128, HC(k), HC(m), 128] of W.T
        return np.ascontiguousarray(
            w.T.reshape(HC, 128, HC, 128).transpose(1, 0, 2, 3))

    wt = lhs_blocks(np.asarray(Wt, np.float32)).astype(BF16)
    w1 = lhs_blocks(np.asarray(W1, np.float32)).astype(BF16)
    w2 = lhs_blocks(np.asarray(W2, np.float32)).astype(BF16)
    wd = np.ascontiguousarray(
        np.asarray(Wd, np.float32).T.reshape(HC, 128, H).transpose(1, 0, 2)).astype(BF16)

    def wd3_block(k):  # [128, 3(hp), HC(m), 2, 128] fp8, DoubleRow pairing
        blk = np.ascontiguousarray(
            np.asarray(Wd3, np.float32)[:, k * H:(k + 1) * H].T
            .reshape(HC, 128, HC, 128).transpose(1, 0, 2, 3))
        blk = blk.reshape(128, 3, 2, HC, 128).transpose(0, 1, 3, 2, 4)
        return _clip8(np.ascontiguousarray(blk), SCW)

    wd3 = np.stack([wd3_block(k) for k in range(3)])
    biases = np.stack([
        np.asarray(v, np.float32).reshape(HC, 128).T
        for v in (bt, np.asarray(bd, np.float32) * SCK, bd3,
                  np.asarray(b1, np.float32) + np.asarray(b2, np.float32))
    ], axis=1)  # [128, 4, HC]
    biases = np.ascontiguousarray(biases, np.float32)
    return wt, wd, wd3, w1, w2, biases


def kernel(**inputs):
    from concourse.bass_utils import run_bass_kernel_spmd

    p = np.asarray(inputs["p"], np.float32)
    option_len = np.asarray(inputs["option_len"]).astype(np.int64)
    lens = (option_len + 1).astype(np.int64)  # [B] key lengths
    glens = lens.reshape(B // 4, 4)

    slots = _assign_groups(glens)  # [GPC][8] group ids
    slot_lens = tuple(
        tuple(int(glens[slots[g]].max(axis=0)[o]) for o in range(4))
        for g in range(GPC))

    if slot_lens not in _GRAPH_CACHE:
        _GRAPH_CACHE[slot_lens] = _build_graph(slot_lens)
    nc = _GRAPH_CACHE[slot_lens]

    wt, wd, wd3, w1, w2, biases = _pack_weights(
        inputs["Wt"], inputs["bt"], inputs["Wd"], inputs["bd"],
        inputs["Wd3"], inputs["bd3"], inputs["W1"], inputs["b1"],
        inputs["W2"], inputs["b2"])

    in_maps = []
    core_groups = []  # [core][g] -> group id
    for c in range(N_CORES):
        gids = [slots[g][c] for g in range(GPC)]
        core_groups.append(gids)
        opts = np.concatenate([np.arange(4) + 4 * gid for gid in gids])
        pc = p[opts]  # [16, S, H]
        pT = pc.transpose(0, 2, 1).reshape(GPC, 4, HC, 128, S)
        pT = np.ascontiguousarray(pT.transpose(0, 2, 3, 1, 4))  # [g, hc, p, o, s]
        maskrow = np.zeros((GPC, 1, 4, S + 1), np.float32)
        for g in range(GPC):
            for o in range(4):
                lc = int(glens[gids[g]][o])
                sl = int(slot_lens[g][o])
                maskrow[g, 0, o, lc:sl] = -30000.0
                maskrow[g, 0, o, sl] = math.log(1e-13 * (S - lc))
        maskbc = np.broadcast_to(maskrow, (GPC, 128, 4, S + 1))
        in_maps.append({
            "p_bf": pT.astype(BF16),
            "maskbc": np.ascontiguousarray(maskbc).astype(BF16),
            "wt": wt, "wd": wd, "wd3": wd3, "w1": w1, "w2": w2,
            "biases": biases,
        })

    try:
        res = run_bass_kernel_spmd(nc, in_maps, list(range(N_CORES)))
    except Exception:
        # a previously wedged device surfaces on the first execute; the
        # runtime resets it, so a single retry suffices
        res = run_bass_kernel_spmd(nc, in_maps, list(range(N_CORES)))

    out = np.empty((B, S, H), np.float32)
    for c in range(N_CORES):
        oc = res.results[c]["out"]  # [GPC, 4, HC, 128, S]
        for g in range(GPC):
            gid = core_groups[c][g]
            # [4, HC, 128, S] -> [4, S, H]
            blk = oc[g].transpose(0, 3, 1, 2).reshape(4, S, H)
            out[4 * gid: 4 * gid + 4] = blk
    return out



# revision 4
# speedup vs baseline: 1.1937x; 1.1937x over previous
"""Trainium2 Bass kernel for the AOI (attention-over-interactions) module.

Strategy (8 NeuronCores, data-parallel over question groups):
  - B=128 options = 32 self-contained groups of 4; 4 "group slots" x 8 cores.
  - Host anneals the group->slot assignment to minimize the baked
    max-over-cores ragged lengths (weighted by the PE cycle model).
  - Constant-shift softmax: scores get a host-baked additive mask of
    -C (in range) / -30000 (padding) / log(1e-13*(S-l))-C (epsilon
    column), which reproduces the reference masked-softmax exactly
    without any per-row max reduction.
  - Mixed precision: bf16 attention path; fp8 (DoubleRow) for the
    Wd3, W1@oc and W2@p matmuls, fp32 accumulation everywhere.
  - Emission is software-pipelined: score->softmax->transpose->av
    chains run 3 deep with trans_q/td/W2@p matmuls of the next group
    interleaved as filler so the tensor engine never idles (keeps the
    PE p-state clock at max).
"""

import math
import sys

for _p in ("/opt/trn_rl_repo", "/opt/pypackages"):
    if _p not in sys.path:
        sys.path.append(_p)

import numpy as np
import ml_dtypes

B, S, H = 128, 256, 768
N_CORES = 8
GPC = 4           # group slots per core
HC = H // 128     # 6 h-chunks
BF16 = ml_dtypes.bfloat16
FP8 = ml_dtypes.float8_e4m3   # TRN float8e4; max 240, overflow -> inf
SCW = 2048.0      # fp8 scale for Wd3
SCK = 64.0        # fp8 scale for ock
SCO = 16.0        # fp8 scale for ocT8 (oc re-quantized for W1@oc)
SCP = 32.0        # fp8 scale for p (W2@p)
SCW1 = 2048.0     # fp8 scale for W1
SCW2 = 2048.0     # fp8 scale for W2
CSH = 30.0        # constant softmax shift

OTHERS = [[1, 2, 3], [0, 2, 3], [0, 1, 3], [0, 1, 2]]


def _clip8(x, scale):
    return np.clip(np.asarray(x, np.float32) * scale, -224.0, 224.0).astype(FP8)

_GRAPH_CACHE = {}


def _assign_groups(glens):
    """Partition 32 groups into GPC slots of 8, minimizing the PE-cycle
    cost of the per-(slot,option) max lengths. Returns [GPC][8] group ids."""
    rng = np.random.default_rng(0)
    n_groups = glens.shape[0]

    def cost(assign):
        c = 0.0
        for ids in assign:
            mx = glens[ids].max(axis=0)
            c += float((72 * mx + 9984 * np.ceil(mx / 128.0)).sum())
        return c

    best, best_cost = None, None
    for seed in range(2):
        r = np.random.default_rng(seed)
        order = np.argsort(-glens.sum(axis=1))
        assign = [list(order[g * 8:(g + 1) * 8]) for g in range(GPC)]
        cur = cost(assign)
        iters = 60000
        for it in range(iters):
            T = 3000.0 * (1 - it / iters) + 1.0
            ga, gb = r.integers(0, GPC), r.integers(0, GPC)
            if ga == gb:
                continue
            ia, ib = r.integers(0, 8), r.integers(0, 8)
            assign[ga][ia], assign[gb][ib] = assign[gb][ib], assign[ga][ia]
            nc_ = cost(assign)
            if nc_ <= cur or r.random() < math.exp((cur - nc_) / T):
                cur = nc_
                if best_cost is None or cur < best_cost:
                    best_cost, best = cur, [list(a) for a in assign]
            else:
                assign[ga][ia], assign[gb][ib] = assign[gb][ib], assign[ga][ia]
    return best


def _build_graph(slot_lens, zero_bias):
    """Build + compile the SPMD Bacc graph. slot_lens: [GPC][4] ints (1..255).

    zero_bias: bd (av bias) is all-zero, enabling merged multi-hc ock
    stores (bias would differ per 128-block of h otherwise)."""
    import concourse.bass as bass
    import concourse.bacc as bacc
    import concourse.mybir as mybir
    import concourse.tile as tile
    from concourse.masks import make_identity

    f32 = mybir.dt.float32
    bf = mybir.dt.bfloat16
    f8 = mybir.dt.float8e4
    AF = mybir.ActivationFunctionType
    DR = mybir.MatmulPerfMode.DoubleRow

    nc = bacc.Bacc("TRN2", target_bir_lowering=False, debug=False,
                   num_devices=N_CORES)

    pbf_d = nc.dram_tensor("p_bf", [GPC, 128, HC, 4, S], bf, kind="ExternalInput")
    p8_d = nc.dram_tensor("p_f8", [GPC, 128, 3, 2, 4, S], f8, kind="ExternalInput")
    msk_d = nc.dram_tensor("maskbc", [GPC, 128, 4, S + 1], bf, kind="ExternalInput")
    wt_d = nc.dram_tensor("wt", [128, HC, HC, 128], bf, kind="ExternalInput")
    wd_d = nc.dram_tensor("wd", [128, HC, H], bf, kind="ExternalInput")
    wd3_d = nc.dram_tensor("wd3", [3, 128, 3, HC, 2, 128], f8, kind="ExternalInput")
    w1_d = nc.dram_tensor("w1f8", [128, 3, HC, 2, 128], f8, kind="ExternalInput")
    w2_d = nc.dram_tensor("w2f8", [128, 3, HC, 2, 128], f8, kind="ExternalInput")
    bias_d = nc.dram_tensor("biases", [128, 5, HC], f32, kind="ExternalInput")
    out_d = nc.dram_tensor("out", [GPC, HC, 128, 4 * S], f32, kind="ExternalOutput")

    L = [[int(x) for x in row] for row in slot_lens]
    TCS = [[max(1, math.ceil(l / 128)) for l in row] for row in L]

    with tile.TileContext(nc) as tc:
        with (
            tc.tile_pool(name="const", bufs=1) as constp,
            tc.tile_pool(name="wres", bufs=1) as wres,
            tc.tile_pool(name="gin", bufs=1) as gin,
            tc.tile_pool(name="act", bufs=1) as act,
            tc.tile_pool(name="smp", bufs=1) as smp,
            tc.tile_pool(name="fin", bufs=1) as fin,
            tc.tile_pool(name="psc", bufs=3, space="PSUM") as psc,
            tc.tile_pool(name="pgp", bufs=5, space="PSUM") as pgp,
        ):
            # ---- constants + weights (resident) ------------------------
            ident_f = constp.tile([128, 128], f32)
            make_identity(nc, ident_f[:])
            ident = constp.tile([128, 128], bf)
            nc.vector.tensor_copy(ident[:], ident_f[:])
            biases = constp.tile([128, 5, HC], f32)
            nc.sync.dma_start(biases[:], bias_d.ap())
            bt_b = biases[:, 0, :]
            bd_b = biases[:, 1, :]     # bd * SCK
            bd3_b = biases[:, 2, :]
            b12_b = biases[:, 3, :]    # b1 + b2
            bd3o_b = biases[:, 4, :]   # bd3 * SCO

            wt_sb = wres.tile([128, HC, HC, 128], bf)
            nc.sync.dma_start(wt_sb[:], wt_d.ap())

            # group inputs: double-buffered; group 0 lands via the scalar
            # engine's DMA queue so it runs in parallel with wt on sync
            pbf_t, p8_t, msk_t = [], [], []
            for g in range(GPC):
                pbf_t.append(gin.tile([128, HC, 4, S], bf, tag="pbf", bufs=2,
                                      name=f"pbf{g}"))
                p8_t.append(gin.tile([128, 3, 2, 4, S], f8, tag="p8", bufs=2,
                                     name=f"p8_{g}"))
                msk_t.append(gin.tile([128, 4, S + 1], bf, tag="msk", bufs=2,
                                      name=f"msk{g}"))

            def load_group(g, eng):
                eng.dma_start(pbf_t[g][:], pbf_d.ap()[g])
                eng.dma_start(p8_t[g][:], p8_d.ap()[g])
                eng.dma_start(msk_t[g][:], msk_d.ap()[g])

            load_group(0, nc.scalar)

            wd_sb = wres.tile([128, HC, H], bf)
            nc.sync.dma_start(wd_sb[:], wd_d.ap())
            wd3_sb = []
            for k in range(3):
                t = wres.tile([128, 3, HC, 2, 128], f8, name=f"wd3_{k}")
                nc.sync.dma_start(t[:], wd3_d.ap()[k])
                wd3_sb.append(t)
            w1_sb = wres.tile([128, 3, HC, 2, 128], f8)
            nc.sync.dma_start(w1_sb[:], w1_d.ap())
            w2_sb = wres.tile([128, 3, HC, 2, 128], f8)
            nc.sync.dma_start(w2_sb[:], w2_d.ap())
            load_group(1, nc.sync)

            # ---- per-group work tiles ---------------------------------
            ttT_t = [act.tile([128, HC, 4, S + 1], bf, tag="ttT", bufs=2,
                              name=f"ttT{g}") for g in range(GPC)]
            z2_t = [act.tile([128, HC, 1024], bf, tag="z2", bufs=2,
                             name=f"z2_{g}") for g in range(GPC)]
            td = act.tile([128, 4, 2, H], bf, tag="td")
            ocT = act.tile([128, HC, 4, S], bf, tag="ocT")
            ocT8 = act.tile([128, 3, 2, 4, S], f8, tag="ocT8")

            # ---- emission building blocks -----------------------------
            def ttT_block(g, m, op):
                # trans_q for option pair (2op, 2op+1), h'-chunk m
                ps = pgp.tile([128, 512], f32, tag="gp", name="ttps")
                for oo in range(2):
                    o = 2 * op + oo
                    lo = L[g][o]
                    for hc in range(HC):
                        nc.tensor.matmul(
                            ps[:, oo * 256: oo * 256 + lo],
                            wt_sb[:, hc, m, :],
                            pbf_t[g][:, hc, o, 0:lo],
                            start=(hc == 0), stop=(hc == HC - 1))
                nc.scalar.activation(
                    ttT_t[g][:, m, 2 * op: 2 * op + 2, 0:256], ps[:],
                    AF.Identity, bias=bt_b[:, m:m + 1])

            def ttT_eps(g):
                for o in range(4):
                    nc.vector.memset(ttT_t[g][:, :, o, L[g][o]:L[g][o] + 1], 0.0)

            def td_block(g, o, tcx):
                lo = L[g][o]
                w = min(128, lo - tcx * 128)
                psA = pgp.tile([128, 512], f32, tag="gp", name="tdA")
                psB = pgp.tile([128, 512], f32, tag="gp", name="tdB")
                for hc in range(HC):
                    lhs = pbf_t[g][:, hc, o, tcx * 128: tcx * 128 + w]
                    nc.tensor.matmul(psA[0:w, 0:512], lhs, wd_sb[:, hc, 0:512],
                                     start=(hc == 0), stop=(hc == HC - 1))
                    nc.tensor.matmul(psB[0:w, 0:256], lhs, wd_sb[:, hc, 512:768],
                                     start=(hc == 0), stop=(hc == HC - 1))
                nc.scalar.activation(td[0:w, o, tcx, 0:512], psA[0:w, 0:512],
                                     AF.Copy)
                nc.scalar.activation(td[0:w, o, tcx, 512:768], psB[0:w, 0:256],
                                     AF.Copy)

            def z2_block(g, m):
                # W2 @ p (fp8 DoubleRow), pre-scaled by SCO*SCW1/(SCP*SCW2)
                zsc = (SCO * SCW1) / (SCP * SCW2)
                for ho in range(2):
                    ps = pgp.tile([128, 512], f32, tag="gp", name="z2ps")
                    for hp in range(3):
                        nc.tensor.matmul(
                            ps[:, :], w2_sb[:, hp, m, :, :],
                            p8_t[g][:, hp, :, 2 * ho: 2 * ho + 2, :],
                            start=(hp == 0), stop=(hp == 2), perf_mode=DR)
                    nc.scalar.activation(
                        z2_t[g][:, m, ho * 512:(ho + 1) * 512], ps[:],
                        AF.Identity, scale=zsc)

            # softmax chain state per in-flight pair
            def sc_block(g, i, j):
                lj = L[g][j]
                ps = psc.tile([128, 512], f32, tag="sc", name="scps")
                for m in range(2):
                    for hc in range(HC):
                        nc.tensor.matmul(
                            ps[:, m * 256: m * 256 + lj + 1],
                            pbf_t[g][:, hc, i, m * 128:(m + 1) * 128],
                            ttT_t[g][:, hc, j, 0:lj + 1],
                            start=(hc == 0), stop=(hc == HC - 1))
                # softmax chain (C-shift baked into the mask)
                stats = smp.tile([128, 4], f32, tag="stats", bufs=6,
                                 name="stats")
                e = smp.tile([128, 2, 256], bf, tag="e", bufs=3, name="e")
                sm = smp.tile([128, 2, 256], bf, tag="sm", bufs=4, name="sm")
                for m in range(2):
                    nc.vector.tensor_add(ps[:, m * 256: m * 256 + lj + 1],
                                         ps[:, m * 256: m * 256 + lj + 1],
                                         msk_t[g][:, j, 0:lj + 1])
                for m in range(2):
                    nc.scalar.activation(
                        e[:, m, 0:lj + 1], ps[:, m * 256: m * 256 + lj + 1],
                        AF.Exp, accum_out=stats[:, m:m + 1])
                nc.vector.reciprocal(stats[:, 2:4], stats[:, 0:2])
                for m in range(2):
                    nc.vector.tensor_scalar_mul(sm[:, m, 0:lj], e[:, m, 0:lj],
                                                stats[:, 2 + m:3 + m])
                return sm

            def tr_block(g, j, sm):
                lj = L[g][j]
                smT = smp.tile([128, 2, 256], bf, tag="smT", bufs=3, name="smT")
                for tcx in range(TCS[g][j]):
                    w = min(128, lj - tcx * 128)
                    tp = pgp.tile([128, 256], bf, tag="gp", name="tp")
                    for m in range(2):
                        nc.tensor.matmul(
                            tp[0:w, m * 128:(m + 1) * 128],
                            sm[:, m, tcx * 128: tcx * 128 + w],
                            ident[:], is_transpose=True,
                            start=(m == 0), stop=(m == 1))
                    nc.scalar.activation(smT[0:w, tcx, :], tp[0:w, :], AF.Copy)
                return smT

            def av_block(g, i, j, jr, smT, ock):
                lj = L[g][j]
                io = i % 2
                for hcp in range(3):
                    ps = pgp.tile([128, 512], f32, tag="gp", name="avps")
                    for h2 in range(2):
                        hc = 2 * hcp + h2
                        for tcx in range(TCS[g][j]):
                            w = min(128, lj - tcx * 128)
                            nc.tensor.matmul(
                                ps[:, h2 * 256:(h2 + 1) * 256],
                                td[0:w, j, tcx, hc * 128:(hc + 1) * 128],
                                smT[0:w, tcx, :],
                                start=(tcx == 0), stop=(tcx == TCS[g][j] - 1))
                    if zero_bias:
                        nc.scalar.activation(
                            ock[:, jr, 2 * hcp: 2 * hcp + 2, io, :], ps[:],
                            AF.Relu, scale=SCK)
                    else:
                        for h2 in range(2):
                            hc = 2 * hcp + h2
                            nc.scalar.activation(
                                ock[:, jr, hc, io, :],
                                ps[:, h2 * 256:(h2 + 1) * 256],
                                AF.Relu, bias=bd_b[:, hc:hc + 1], scale=SCK)

            def wd3_mblock(g, half, m, ock):
                ops = pgp.tile([128, 512], f32, tag="gp", name="wd3ps")
                step = 0
                for k in range(3):
                    for hp in range(3):
                        nc.tensor.matmul(
                            ops[:, :], wd3_sb[k][:, hp, m, :, :],
                            ock[:, k, 2 * hp: 2 * hp + 2, :, :],
                            start=(step == 0), stop=(step == 8),
                            perf_mode=DR)
                        step += 1
                nc.scalar.activation(
                    ocT[:, m, 2 * half: 2 * half + 2, :], ops[:],
                    AF.Identity, bias=bd3_b[:, m:m + 1],
                    scale=1.0 / (SCK * SCW))
                nc.scalar.activation(
                    ocT8[:, m // 2, m % 2, 2 * half: 2 * half + 2, :], ops[:],
                    AF.Identity, bias=bd3o_b[:, m:m + 1],
                    scale=SCO / (SCK * SCW))

            def final_mblock(g, m):
                mid = fin.tile([128, 1024], bf, tag="mid", bufs=2, name="mid")
                for ho in range(2):
                    ps = pgp.tile([128, 512], f32, tag="gp", name="z1ps")
                    for hp in range(3):
                        nc.tensor.matmul(
                            ps[:, :], w1_sb[:, hp, m, :, :],
                            ocT8[:, hp, :, 2 * ho: 2 * ho + 2, :],
                            start=(hp == 0), stop=(hp == 2), perf_mode=DR)
                    nc.vector.tensor_add(ps[:, :], ps[:, :],
                                         z2_t[g][:, m, ho * 512:(ho + 1) * 512])
                    nc.scalar.activation(
                        mid[:, ho * 512:(ho + 1) * 512], ps[:], AF.Sigmoid,
                        bias=b12_b[:, m:m + 1], scale=1.0 / (SCO * SCW1))
                fb = fin.tile([128, 1024], f32, tag="fb", bufs=2, name="fb")
                nc.gpsimd.tensor_sub(fb[:], pbf_t[g][:, m, :, :],
                                     ocT[:, m, :, :])
                nc.vector.tensor_mul(fb[:], fb[:], mid[:])
                nc.vector.tensor_add(fb[:], fb[:], ocT[:, m, :, :])
                nc.sync.dma_start(out_d.ap()[g][m], fb[:])

            # ---- schedule ---------------------------------------------
            # group 0 prologue: trans_q + td + z2, pure PE burst
            for m in range(HC):
                for op in range(2):
                    ttT_block(0, m, op)
            ttT_eps(0)
            for o in range(4):
                for tcx in range(TCS[0][o]):
                    td_block(0, o, tcx)
            for m in range(HC):
                z2_block(0, m)

            for g in range(GPC):
                # filler reservoir: next group's trans_q + z2 blocks
                fillers = []
                if g + 1 < GPC:
                    fillers += [(ttT_block, (g + 1, m, op))
                                for m in range(HC) for op in range(2)]
                    fillers += [(z2_block, (g + 1, m)) for m in range(HC)]

                def pop_filler(n):
                    for _ in range(n):
                        if fillers:
                            f, a = fillers.pop(0)
                            f(*a)

                ock_t = [act.tile([128, 3, HC, 2, S], f8, tag="ock", bufs=2,
                                  name=f"ock{g}h{h}") for h in range(2)]
                pairs = [(i, j) for i in range(4) for j in OTHERS[i]]
                sms = [None] * 12
                # score prologue, 3 chains deep
                for p in range(3):
                    sms[p] = sc_block(g, *pairs[p])
                for p in range(12):
                    i, j = pairs[p]
                    smT = tr_block(g, j, sms[p])
                    sms[p] = None
                    if p + 3 < 12:
                        sms[p + 3] = sc_block(g, *pairs[p + 3])
                    av_block(g, i, j, OTHERS[i].index(j), smT, ock_t[i // 2])
                    if 6 <= p:
                        wd3_mblock(g, 0, p - 6, ock_t[0])
                    if p >= 4:
                        pop_filler(1)
                # half-1 Wd3 + epilogue
                for m in range(HC):
                    wd3_mblock(g, 1, m, ock_t[1])
                for m in range(HC):
                    final_mblock(g, m)
                # g+2 reuses g's input buffers: emit after final's reads
                if g + 2 < GPC:
                    load_group(g + 2, nc.sync)
                # remaining fillers + next group's td (td has one buffer:
                # its av readers for group g are all emitted above)
                pop_filler(len(fillers))
                if g + 1 < GPC:
                    ttT_eps(g + 1)
                    for o in range(4):
                        for tcx in range(TCS[g + 1][o]):
                            td_block(g + 1, o, tcx)

    nc.compile()
    return nc


def _lhs_blocks(w):  # [H,H] -> [128, HC(k), HC(m), 128] of W.T
    return np.ascontiguousarray(
        np.asarray(w, np.float32).T.reshape(HC, 128, HC, 128)
        .transpose(1, 0, 2, 3))


def _dr_blocks8(w, scale):
    # [H,H] -> [128, 3(hp), HC(m), 2(dr), 128] fp8, DoubleRow pairing
    blk = _lhs_blocks(w)                       # [128, 6(k), 6(m), 128]
    blk = blk.reshape(128, 3, 2, HC, 128).transpose(0, 1, 3, 2, 4)
    return _clip8(np.ascontiguousarray(blk), scale)


def _pack_weights(Wt, bt, Wd, bd, Wd3, bd3, W1, b1, W2, b2):
    wt = _lhs_blocks(Wt).astype(BF16)
    wd = np.ascontiguousarray(
        np.asarray(Wd, np.float32).T.reshape(HC, 128, H).transpose(1, 0, 2)).astype(BF16)

    def wd3_block(k):
        blk = np.ascontiguousarray(
            np.asarray(Wd3, np.float32)[:, k * H:(k + 1) * H].T
            .reshape(HC, 128, HC, 128).transpose(1, 0, 2, 3))
        blk = blk.reshape(128, 3, 2, HC, 128).transpose(0, 1, 3, 2, 4)
        return _clip8(np.ascontiguousarray(blk), SCW)

    wd3 = np.stack([wd3_block(k) for k in range(3)])
    w1f8 = _dr_blocks8(W1, SCW1)
    w2f8 = _dr_blocks8(W2, SCW2)
    bd3f = np.asarray(bd3, np.float32)
    biases = np.stack([
        np.asarray(v, np.float32).reshape(HC, 128).T
        for v in (bt, np.asarray(bd, np.float32) * SCK, bd3f,
                  np.asarray(b1, np.float32) + np.asarray(b2, np.float32),
                  bd3f * SCO)
    ], axis=1)  # [128, 5, HC]
    biases = np.ascontiguousarray(biases, np.float32)
    return wt, wd, wd3, w1f8, w2f8, biases


def kernel(**inputs):
    from concourse.bass_utils import run_bass_kernel_spmd

    p = np.asarray(inputs["p"], np.float32)
    option_len = np.asarray(inputs["option_len"]).astype(np.int64)
    lens = (option_len + 1).astype(np.int64)  # [B] key lengths
    glens = lens.reshape(B // 4, 4)

    slots = _assign_groups(glens)  # [GPC][8] group ids
    slot_lens = tuple(
        tuple(int(glens[slots[g]].max(axis=0)[o]) for o in range(4))
        for g in range(GPC))
    zero_bias = not np.any(np.asarray(inputs["bd"], np.float32))

    key = (slot_lens, zero_bias)
    if key not in _GRAPH_CACHE:
        _GRAPH_CACHE[key] = _build_graph(slot_lens, zero_bias)
    nc = _GRAPH_CACHE[key]

    wt, wd, wd3, w1f8, w2f8, biases = _pack_weights(
        inputs["Wt"], inputs["bt"], inputs["Wd"], inputs["bd"],
        inputs["Wd3"], inputs["bd3"], inputs["W1"], inputs["b1"],
        inputs["W2"], inputs["b2"])

    in_maps = []
    core_groups = []  # [core][g] -> group id
    for c in range(N_CORES):
        gids = [slots[g][c] for g in range(GPC)]
        core_groups.append(gids)
        opts = np.concatenate([np.arange(4) + 4 * gid for gid in gids])
        pc = p[opts]  # [16, S, H]
        # [g, o, s, h] -> partition-major layouts
        pg = pc.reshape(GPC, 4, S, H)
        # pbf: [g, 128(part), hc, o, s]  (h = hc*128 + part)
        pT = pg.transpose(0, 3, 1, 2).reshape(GPC, HC, 128, 4, S)
        pbf = np.ascontiguousarray(pT.transpose(0, 2, 1, 3, 4)).astype(BF16)
        # p8: [g, 128(part), hp, dr, o, s]  (h = hp*256 + dr*128 + part)
        p8v = pg.transpose(0, 3, 1, 2).reshape(GPC, 3, 2, 128, 4, S)
        p8 = _clip8(np.ascontiguousarray(p8v.transpose(0, 3, 1, 2, 4, 5)), SCP)
        maskrow = np.zeros((GPC, 1, 4, S + 1), np.float32)
        for g in range(GPC):
            for o in range(4):
                lc = int(glens[gids[g]][o])
                sl = int(slot_lens[g][o])
                maskrow[g, 0, o, 0:lc] = -CSH
                maskrow[g, 0, o, lc:sl] = -30000.0
                maskrow[g, 0, o, sl] = math.log(1e-13 * (S - lc)) - CSH
        maskbc = np.broadcast_to(maskrow, (GPC, 128, 4, S + 1))
        in_maps.append({
            "p_bf": pbf,
            "p_f8": p8,
            "maskbc": np.ascontiguousarray(maskbc).astype(BF16),
            "wt": wt, "wd": wd, "wd3": wd3, "w1f8": w1f8, "w2f8": w2f8,
            "biases": biases,
        })

    try:
        res = run_bass_kernel_spmd(nc, in_maps, list(range(N_CORES)))
    except Exception:
        # a previously wedged device surfaces on the first execute; the
        # runtime resets it, so a single retry suffices
        res = run_bass_kernel_spmd(nc, in_maps, list(range(N_CORES)))

    out = np.empty((B, S, H), np.float32)
    for c in range(N_CORES):
        oc = res.results[c]["out"]  # [GPC, HC, 128, 4*S]
        for g in range(GPC):
            gid = core_groups[c][g]
            # [HC, 128, 4, S] -> [4, S, H]
            blk = (oc[g].reshape(HC, 128, 4, S).transpose(2, 3, 0, 1)
                   .reshape(4, S, H))
            out[4 * gid: 4 * gid + 4] = blk
    return out


# revision 5
# speedup vs baseline: 1.2657x; 1.0603x over previous
"""Trainium2 Bass kernel for the AOI (attention-over-interactions) module.

Strategy (8 NeuronCores, data-parallel over question groups):
  - B=128 options = 32 self-contained groups of 4; 4 "group slots" x 8 cores.
  - Host anneals the group->slot assignment to minimize the baked
    max-over-cores ragged lengths (weighted by the PE cycle model).
  - Constant-shift softmax: scores get a host-baked additive mask of
    -C (in range) / -30000 (padding) / log(1e-13*(S-l))-C (epsilon
    column), which reproduces the reference masked-softmax exactly
    without any per-row max reduction.
  - Mixed precision: bf16 attention path; fp8 (DoubleRow) for the
    Wd3, W1@oc and W2@p matmuls, fp32 accumulation everywhere.
  - Emission is software-pipelined: score->softmax->transpose->av
    chains run 3 deep with trans_q/td/W2@p matmuls of the next group
    interleaved as filler so the tensor engine never idles (keeps the
    PE p-state clock at max).
"""

import math
import sys

for _p in ("/opt/trn_rl_repo", "/opt/pypackages"):
    if _p not in sys.path:
        sys.path.append(_p)

import numpy as np
import ml_dtypes

B, S, H = 128, 256, 768
N_CORES = 8
GPC = 4           # group slots per core
HC = H // 128     # 6 h-chunks
BF16 = ml_dtypes.bfloat16
FP8 = ml_dtypes.float8_e4m3   # TRN float8e4; max 240, overflow -> inf
SCW = 2048.0      # fp8 scale for Wd3
SCK = 64.0        # fp8 scale for ock
SCO = 16.0        # fp8 scale for ocT8 (oc re-quantized for W1@oc)
SCP = 32.0        # fp8 scale for p (W2@p)
SCW1 = 2048.0     # fp8 scale for W1
SCW2 = 2048.0     # fp8 scale for W2
CSH = 30.0        # constant softmax shift

OTHERS = [[1, 2, 3], [0, 2, 3], [0, 1, 3], [0, 1, 2]]


def _clip8(x, scale):
    return np.clip(np.asarray(x, np.float32) * scale, -224.0, 224.0).astype(FP8)

_GRAPH_CACHE = {}


def _assign_groups(glens):
    """Partition 32 groups into GPC slots of 8, minimizing the PE-cycle
    cost of the per-(slot,option) max lengths. Returns [GPC][8] group ids."""
    rng = np.random.default_rng(0)
    n_groups = glens.shape[0]

    def cost(assign):
        c = 0.0
        for ids in assign:
            mx = glens[ids].max(axis=0)
            c += float((72 * mx + 9984 * np.ceil(mx / 128.0)).sum())
        return c

    best, best_cost = None, None
    for seed in range(2):
        r = np.random.default_rng(seed)
        order = np.argsort(-glens.sum(axis=1))
        assign = [list(order[g * 8:(g + 1) * 8]) for g in range(GPC)]
        cur = cost(assign)
        iters = 60000
        for it in range(iters):
            T = 3000.0 * (1 - it / iters) + 1.0
            ga, gb = r.integers(0, GPC), r.integers(0, GPC)
            if ga == gb:
                continue
            ia, ib = r.integers(0, 8), r.integers(0, 8)
            assign[ga][ia], assign[gb][ib] = assign[gb][ib], assign[ga][ia]
            nc_ = cost(assign)
            if nc_ <= cur or r.random() < math.exp((cur - nc_) / T):
                cur = nc_
                if best_cost is None or cur < best_cost:
                    best_cost, best = cur, [list(a) for a in assign]
            else:
                assign[ga][ia], assign[gb][ib] = assign[gb][ib], assign[ga][ia]
    return best


def _build_graph(slot_lens, zero_bias):
    """Build + compile the SPMD Bacc graph. slot_lens: [GPC][4] ints (1..255).

    zero_bias: bd (av bias) is all-zero, enabling merged multi-hc ock
    stores (bias would differ per 128-block of h otherwise)."""
    import concourse.bass as bass
    import concourse.bacc as bacc
    import concourse.mybir as mybir
    import concourse.tile as tile
    from concourse.masks import make_identity

    f32 = mybir.dt.float32
    bf = mybir.dt.bfloat16
    f8 = mybir.dt.float8e4
    AF = mybir.ActivationFunctionType
    DR = mybir.MatmulPerfMode.DoubleRow

    nc = bacc.Bacc("TRN2", target_bir_lowering=False, debug=False,
                   num_devices=N_CORES)

    pbf_d = nc.dram_tensor("p_bf", [GPC, 128, HC, 4, S], bf, kind="ExternalInput")
    p8_d = nc.dram_tensor("p_f8", [GPC, 128, 3, 2, 4, S], f8, kind="ExternalInput")
    msk_d = nc.dram_tensor("maskbc", [GPC, 128, 4, S + 1], bf, kind="ExternalInput")
    wt_d = nc.dram_tensor("wt", [128, HC, HC, 128], bf, kind="ExternalInput")
    wd_d = nc.dram_tensor("wd", [128, HC, H], bf, kind="ExternalInput")
    wd3_d = nc.dram_tensor("wd3", [3, 128, 3, HC, 2, 128], f8, kind="ExternalInput")
    w1_d = nc.dram_tensor("w1f8", [128, 3, HC, 2, 128], f8, kind="ExternalInput")
    w2_d = nc.dram_tensor("w2f8", [128, 3, HC, 2, 128], f8, kind="ExternalInput")
    bias_d = nc.dram_tensor("biases", [128, 5, HC], f32, kind="ExternalInput")
    out_d = nc.dram_tensor("out", [GPC, HC, 128, 4 * S], bf, kind="ExternalOutput")

    L = [[int(x) for x in row] for row in slot_lens]
    TCS = [[max(1, math.ceil(l / 128)) for l in row] for row in L]

    with tile.TileContext(nc) as tc:
        with (
            tc.tile_pool(name="const", bufs=1) as constp,
            tc.tile_pool(name="wres", bufs=1) as wres,
            tc.tile_pool(name="gin", bufs=1) as gin,
            tc.tile_pool(name="act", bufs=1) as act,
            tc.tile_pool(name="smp", bufs=1) as smp,
            tc.tile_pool(name="fin", bufs=1) as fin,
            tc.tile_pool(name="psc", bufs=3, space="PSUM") as psc,
            tc.tile_pool(name="pgp", bufs=5, space="PSUM") as pgp,
        ):
            # ---- constants + weights (resident) ------------------------
            ident_f = constp.tile([128, 128], f32)
            make_identity(nc, ident_f[:])
            ident = constp.tile([128, 128], bf)
            nc.vector.tensor_copy(ident[:], ident_f[:])
            biases = constp.tile([128, 5, HC], f32)
            nc.sync.dma_start(biases[:], bias_d.ap())
            bt_b = biases[:, 0, :]
            bd_b = biases[:, 1, :]     # bd * SCK
            bd3_b = biases[:, 2, :]
            b12_b = biases[:, 3, :]    # b1 + b2
            bd3o_b = biases[:, 4, :]   # bd3 * SCO

            wt_sb = wres.tile([128, HC, HC, 128], bf)
            nc.sync.dma_start(wt_sb[:], wt_d.ap())

            # group inputs: double-buffered; group 0 lands via the scalar
            # engine's DMA queue so it runs in parallel with wt on sync
            pbf_t, p8_t, msk_t = [], [], []
            for g in range(GPC):
                pbf_t.append(gin.tile([128, HC, 4, S], bf, tag="pbf", bufs=2,
                                      name=f"pbf{g}"))
                p8_t.append(gin.tile([128, 3, 2, 4, S], f8, tag="p8", bufs=2,
                                     name=f"p8_{g}"))
                msk_t.append(gin.tile([128, 4, S + 1], bf, tag="msk", bufs=2,
                                      name=f"msk{g}"))

            def load_group(g, eng):
                eng.dma_start(pbf_t[g][:], pbf_d.ap()[g])
                eng.dma_start(p8_t[g][:], p8_d.ap()[g])
                eng.dma_start(msk_t[g][:], msk_d.ap()[g])

            load_group(0, nc.scalar)

            wd_sb = wres.tile([128, HC, H], bf)
            nc.sync.dma_start(wd_sb[:], wd_d.ap())
            wd3_sb = []
            for k in range(3):
                t = wres.tile([128, 3, HC, 2, 128], f8, name=f"wd3_{k}")
                nc.sync.dma_start(t[:], wd3_d.ap()[k])
                wd3_sb.append(t)
            w1_sb = wres.tile([128, 3, HC, 2, 128], f8)
            nc.sync.dma_start(w1_sb[:], w1_d.ap())
            w2_sb = wres.tile([128, 3, HC, 2, 128], f8)
            nc.sync.dma_start(w2_sb[:], w2_d.ap())
            load_group(1, nc.sync)

            # ---- per-group work tiles ---------------------------------
            ttT_t = [act.tile([128, HC, 4, S + 1], bf, tag="ttT", bufs=2,
                              name=f"ttT{g}") for g in range(GPC)]
            z2_t = [act.tile([128, HC, 1024], bf, tag="z2", bufs=2,
                             name=f"z2_{g}") for g in range(GPC)]
            td = act.tile([128, 4, 2, H], bf, tag="td")
            ocT = act.tile([128, HC, 4, S], bf, tag="ocT")
            ocT8 = act.tile([128, 3, 2, 4, S], f8, tag="ocT8")

            # ---- emission building blocks -----------------------------
            def ttT_block(g, m, op):
                # trans_q for option pair (2op, 2op+1), h'-chunk m
                ps = pgp.tile([128, 512], f32, tag="gp", name="ttps")
                for oo in range(2):
                    o = 2 * op + oo
                    lo = L[g][o]
                    for hc in range(HC):
                        nc.tensor.matmul(
                            ps[:, oo * 256: oo * 256 + lo],
                            wt_sb[:, hc, m, :],
                            pbf_t[g][:, hc, o, 0:lo],
                            start=(hc == 0), stop=(hc == HC - 1))
                if zero_bias:
                    nc.scalar.activation(
                        ttT_t[g][:, m, 2 * op: 2 * op + 2, 0:256], ps[:],
                        AF.Identity)
                else:
                    nc.scalar.activation(
                        ttT_t[g][:, m, 2 * op: 2 * op + 2, 0:256], ps[:],
                        AF.Identity, bias=bt_b[:, m:m + 1])

            def ttT_eps(g):
                for o in range(4):
                    nc.vector.memset(ttT_t[g][:, :, o, L[g][o]:L[g][o] + 1], 0.0)

            def td_block(g, o, tcx):
                lo = L[g][o]
                w = min(128, lo - tcx * 128)
                psA = pgp.tile([128, 512], f32, tag="gp", name="tdA")
                psB = pgp.tile([128, 512], f32, tag="gp", name="tdB")
                for hc in range(HC):
                    lhs = pbf_t[g][:, hc, o, tcx * 128: tcx * 128 + w]
                    nc.tensor.matmul(psA[0:w, 0:512], lhs, wd_sb[:, hc, 0:512],
                                     start=(hc == 0), stop=(hc == HC - 1))
                    nc.tensor.matmul(psB[0:w, 0:256], lhs, wd_sb[:, hc, 512:768],
                                     start=(hc == 0), stop=(hc == HC - 1))
                nc.vector.tensor_copy(td[0:w, o, tcx, 0:512], psA[0:w, 0:512])
                nc.vector.tensor_copy(td[0:w, o, tcx, 512:768], psB[0:w, 0:256])

            def z2_block(g, m):
                # W2 @ p (fp8 DoubleRow), pre-scaled by SCO*SCW1/(SCP*SCW2)
                zsc = (SCO * SCW1) / (SCP * SCW2)
                for ho in range(2):
                    ps = pgp.tile([128, 512], f32, tag="gp", name="z2ps")
                    for hp in range(3):
                        nc.tensor.matmul(
                            ps[:, :], w2_sb[:, hp, m, :, :],
                            p8_t[g][:, hp, :, 2 * ho: 2 * ho + 2, :],
                            start=(hp == 0), stop=(hp == 2), perf_mode=DR)
                    nc.vector.tensor_scalar_mul(
                        z2_t[g][:, m, ho * 512:(ho + 1) * 512], ps[:], zsc)

            # softmax chain state per in-flight pair
            def sc_block(g, i, j):
                lj = L[g][j]
                ps = psc.tile([128, 512], f32, tag="sc", name="scps")
                for m in range(2):
                    for hc in range(HC):
                        nc.tensor.matmul(
                            ps[:, m * 256: m * 256 + lj + 1],
                            pbf_t[g][:, hc, i, m * 128:(m + 1) * 128],
                            ttT_t[g][:, hc, j, 0:lj + 1],
                            start=(hc == 0), stop=(hc == HC - 1))
                # softmax chain (C-shift baked into the mask)
                stats = smp.tile([128, 4], f32, tag="stats", bufs=6,
                                 name="stats")
                e = smp.tile([128, 2, 256], bf, tag="e", bufs=3, name="e")
                sm = smp.tile([128, 2, 256], bf, tag="sm", bufs=4, name="sm")
                for m in range(2):
                    nc.vector.tensor_add(ps[:, m * 256: m * 256 + lj + 1],
                                         ps[:, m * 256: m * 256 + lj + 1],
                                         msk_t[g][:, j, 0:lj + 1])
                for m in range(2):
                    nc.scalar.activation(
                        e[:, m, 0:lj + 1], ps[:, m * 256: m * 256 + lj + 1],
                        AF.Exp, accum_out=stats[:, m:m + 1])
                nc.vector.reciprocal(stats[:, 2:4], stats[:, 0:2])
                for m in range(2):
                    nc.scalar.activation(sm[:, m, 0:lj], e[:, m, 0:lj],
                                         AF.Copy,
                                         scale=stats[:, 2 + m:3 + m])
                return sm

            def tr_block(g, j, sm):
                lj = L[g][j]
                smT = smp.tile([128, 2, 256], bf, tag="smT", bufs=3, name="smT")
                for tcx in range(TCS[g][j]):
                    w = min(128, lj - tcx * 128)
                    tp = pgp.tile([128, 256], bf, tag="gp", name="tp")
                    for m in range(2):
                        nc.tensor.matmul(
                            tp[0:w, m * 128:(m + 1) * 128],
                            sm[:, m, tcx * 128: tcx * 128 + w],
                            ident[:], is_transpose=True,
                            start=(m == 0), stop=(m == 1))
                    nc.scalar.activation(smT[0:w, tcx, :], tp[0:w, :], AF.Copy)
                return smT

            def av_block(g, i, j, jr, smT, ock):
                lj = L[g][j]
                io = i % 2
                for hcp in range(3):
                    ps = pgp.tile([128, 512], f32, tag="gp", name="avps")
                    for h2 in range(2):
                        hc = 2 * hcp + h2
                        for tcx in range(TCS[g][j]):
                            w = min(128, lj - tcx * 128)
                            nc.tensor.matmul(
                                ps[:, h2 * 256:(h2 + 1) * 256],
                                td[0:w, j, tcx, hc * 128:(hc + 1) * 128],
                                smT[0:w, tcx, :],
                                start=(tcx == 0), stop=(tcx == TCS[g][j] - 1))
                    if zero_bias:
                        dst = ock[:, jr, 2 * hcp: 2 * hcp + 2, io, :]
                        if hcp == 2:
                            nc.scalar.activation(dst, ps[:], AF.Relu, scale=SCK)
                        else:
                            nc.vector.tensor_scalar(
                                dst, ps[:], SCK, 0.0,
                                op0=mybir.AluOpType.mult,
                                op1=mybir.AluOpType.max)
                    else:
                        for h2 in range(2):
                            hc = 2 * hcp + h2
                            nc.scalar.activation(
                                ock[:, jr, hc, io, :],
                                ps[:, h2 * 256:(h2 + 1) * 256],
                                AF.Relu, bias=bd_b[:, hc:hc + 1], scale=SCK)

            def wd3_mblock(g, half, m, ock):
                ops = pgp.tile([128, 512], f32, tag="gp", name="wd3ps")
                step = 0
                for k in range(3):
                    for hp in range(3):
                        nc.tensor.matmul(
                            ops[:, :], wd3_sb[k][:, hp, m, :, :],
                            ock[:, k, 2 * hp: 2 * hp + 2, :, :],
                            start=(step == 0), stop=(step == 8),
                            perf_mode=DR)
                        step += 1
                if zero_bias:
                    nc.scalar.activation(
                        ocT[:, m, 2 * half: 2 * half + 2, :], ops[:],
                        AF.Identity, scale=1.0 / (SCK * SCW))
                    nc.vector.tensor_scalar_mul(
                        ocT8[:, m // 2, m % 2, 2 * half: 2 * half + 2, :],
                        ops[:], SCO / (SCK * SCW))
                else:
                    nc.scalar.activation(
                        ocT[:, m, 2 * half: 2 * half + 2, :], ops[:],
                        AF.Identity, bias=bd3_b[:, m:m + 1],
                        scale=1.0 / (SCK * SCW))
                    nc.scalar.activation(
                        ocT8[:, m // 2, m % 2, 2 * half: 2 * half + 2, :],
                        ops[:], AF.Identity, bias=bd3o_b[:, m:m + 1],
                        scale=SCO / (SCK * SCW))

            def final_mblock(g, m):
                mid = fin.tile([128, 1024], bf, tag="mid", bufs=2, name="mid")
                for ho in range(2):
                    ps = pgp.tile([128, 512], f32, tag="gp", name="z1ps")
                    for hp in range(3):
                        nc.tensor.matmul(
                            ps[:, :], w1_sb[:, hp, m, :, :],
                            ocT8[:, hp, :, 2 * ho: 2 * ho + 2, :],
                            start=(hp == 0), stop=(hp == 2), perf_mode=DR)
                    nc.vector.tensor_add(ps[:, :], ps[:, :],
                                         z2_t[g][:, m, ho * 512:(ho + 1) * 512])
                    if zero_bias:
                        nc.scalar.activation(
                            mid[:, ho * 512:(ho + 1) * 512], ps[:], AF.Sigmoid,
                            scale=1.0 / (SCO * SCW1))
                    else:
                        nc.scalar.activation(
                            mid[:, ho * 512:(ho + 1) * 512], ps[:], AF.Sigmoid,
                            bias=b12_b[:, m:m + 1], scale=1.0 / (SCO * SCW1))
                fb = fin.tile([128, 1024], bf, tag="fb", bufs=3, name="fb")
                nc.gpsimd.tensor_sub(fb[:], pbf_t[g][:, m, :, :],
                                     ocT[:, m, :, :])
                nc.vector.tensor_mul(fb[:], fb[:], mid[:])
                nc.vector.tensor_add(fb[:], fb[:], ocT[:, m, :, :])
                (nc.sync if m % 2 == 0 else nc.scalar).dma_start(
                    out_d.ap()[g][m], fb[:])

            # ---- schedule ---------------------------------------------
            # group 0 prologue: trans_q + td + z2, pure PE burst
            for m in range(HC):
                for op in range(2):
                    ttT_block(0, m, op)
            ttT_eps(0)
            for o in range(4):
                for tcx in range(TCS[0][o]):
                    td_block(0, o, tcx)
            for m in range(HC):
                z2_block(0, m)

            for g in range(GPC):
                # filler reservoir: next group's trans_q + z2 blocks
                fillers = []
                if g + 1 < GPC:
                    fillers += [(ttT_block, (g + 1, m, op))
                                for m in range(HC) for op in range(2)]
                    fillers += [(z2_block, (g + 1, m)) for m in range(HC)]

                def pop_filler(n):
                    for _ in range(n):
                        if fillers:
                            f, a = fillers.pop(0)
                            f(*a)

                ock_t = [act.tile([128, 3, HC, 2, S], f8, tag="ock", bufs=2,
                                  name=f"ock{g}h{h}") for h in range(2)]
                pairs = [(i, j) for i in range(4) for j in OTHERS[i]]
                sms = [None] * 12
                # score prologue, 3 chains deep
                for p in range(3):
                    sms[p] = sc_block(g, *pairs[p])
                for p in range(12):
                    i, j = pairs[p]
                    smT = tr_block(g, j, sms[p])
                    sms[p] = None
                    if p + 3 < 12:
                        sms[p + 3] = sc_block(g, *pairs[p + 3])
                    av_block(g, i, j, OTHERS[i].index(j), smT, ock_t[i // 2])
                    if 6 <= p:
                        wd3_mblock(g, 0, p - 6, ock_t[0])
                    if p >= 4:
                        pop_filler(1)
                # half-1 Wd3 + epilogue
                for m in range(HC):
                    wd3_mblock(g, 1, m, ock_t[1])
                for m in range(HC):
                    final_mblock(g, m)
                # g+2 reuses g's input buffers: emit after final's reads
                if g + 2 < GPC:
                    load_group(g + 2, nc.sync)
                # remaining fillers + next group's td (td has one buffer:
                # its av readers for group g are all emitted above)
                pop_filler(len(fillers))
                if g + 1 < GPC:
                    ttT_eps(g + 1)
                    for o in range(4):
                        for tcx in range(TCS[g + 1][o]):
                            td_block(g + 1, o, tcx)

    nc.compile()
    return nc


def _lhs_blocks(w):  # [H,H] -> [128, HC(k), HC(m), 128] of W.T
    return np.ascontiguousarray(
        np.asarray(w, np.float32).T.reshape(HC, 128, HC, 128)
        .transpose(1, 0, 2, 3))


def _dr_blocks8(w, scale):
    # [H,H] -> [128, 3(hp), HC(m), 2(dr), 128] fp8, DoubleRow pairing
    blk = _lhs_blocks(w)                       # [128, 6(k), 6(m), 128]
    blk = blk.reshape(128, 3, 2, HC, 128).transpose(0, 1, 3, 2, 4)
    return _clip8(np.ascontiguousarray(blk), scale)


def _pack_weights(Wt, bt, Wd, bd, Wd3, bd3, W1, b1, W2, b2):
    wt = _lhs_blocks(Wt).astype(BF16)
    wd = np.ascontiguousarray(
        np.asarray(Wd, np.float32).T.reshape(HC, 128, H).transpose(1, 0, 2)).astype(BF16)

    def wd3_block(k):
        blk = np.ascontiguousarray(
            np.asarray(Wd3, np.float32)[:, k * H:(k + 1) * H].T
            .reshape(HC, 128, HC, 128).transpose(1, 0, 2, 3))
        blk = blk.reshape(128, 3, 2, HC, 128).transpose(0, 1, 3, 2, 4)
        return _clip8(np.ascontiguousarray(blk), SCW)

    wd3 = np.stack([wd3_block(k) for k in range(3)])
    w1f8 = _dr_blocks8(W1, SCW1)
    w2f8 = _dr_blocks8(W2, SCW2)
    bd3f = np.asarray(bd3, np.float32)
    biases = np.stack([
        np.asarray(v, np.float32).reshape(HC, 128).T
        for v in (bt, np.asarray(bd, np.float32) * SCK, bd3f,
                  np.asarray(b1, np.float32) + np.asarray(b2, np.float32),
                  bd3f * SCO)
    ], axis=1)  # [128, 5, HC]
    biases = np.ascontiguousarray(biases, np.float32)
    return wt, wd, wd3, w1f8, w2f8, biases


def kernel(**inputs):
    from concourse.bass_utils import run_bass_kernel_spmd

    p = np.asarray(inputs["p"], np.float32)
    option_len = np.asarray(inputs["option_len"]).astype(np.int64)
    lens = (option_len + 1).astype(np.int64)  # [B] key lengths
    glens = lens.reshape(B // 4, 4)

    slots = _assign_groups(glens)  # [GPC][8] group ids
    slot_lens = tuple(
        tuple(int(glens[slots[g]].max(axis=0)[o]) for o in range(4))
        for g in range(GPC))
    zero_bias = not any(
        np.any(np.asarray(inputs[k], np.float32))
        for k in ("bt", "bd", "bd3", "b1", "b2"))

    key = (slot_lens, zero_bias)
    if key not in _GRAPH_CACHE:
        _GRAPH_CACHE[key] = _build_graph(slot_lens, zero_bias)
    nc = _GRAPH_CACHE[key]

    wt, wd, wd3, w1f8, w2f8, biases = _pack_weights(
        inputs["Wt"], inputs["bt"], inputs["Wd"], inputs["bd"],
        inputs["Wd3"], inputs["bd3"], inputs["W1"], inputs["b1"],
        inputs["W2"], inputs["b2"])

    in_maps = []
    core_groups = []  # [core][g] -> group id
    for c in range(N_CORES):
        gids = [slots[g][c] for g in range(GPC)]
        core_groups.append(gids)
        opts = np.concatenate([np.arange(4) + 4 * gid for gid in gids])
        pc = p[opts]  # [16, S, H]
        # [g, o, s, h] -> partition-major layouts
        pg = pc.reshape(GPC, 4, S, H)
        # pbf: [g, 128(part), hc, o, s]  (h = hc*128 + part)
        pT = pg.transpose(0, 3, 1, 2).reshape(GPC, HC, 128, 4, S)
        pbf = np.ascontiguousarray(pT.transpose(0, 2, 1, 3, 4)).astype(BF16)
        # p8: [g, 128(part), hp, dr, o, s]  (h = hp*256 + dr*128 + part)
        p8v = pg.transpose(0, 3, 1, 2).reshape(GPC, 3, 2, 128, 4, S)
        p8 = _clip8(np.ascontiguousarray(p8v.transpose(0, 3, 1, 2, 4, 5)), SCP)
        maskrow = np.zeros((GPC, 1, 4, S + 1), np.float32)
        for g in range(GPC):
            for o in range(4):
                lc = int(glens[gids[g]][o])
                sl = int(slot_lens[g][o])
                maskrow[g, 0, o, 0:lc] = -CSH
                maskrow[g, 0, o, lc:sl] = -30000.0
                maskrow[g, 0, o, sl] = math.log(1e-13 * (S - lc)) - CSH
        maskbc = np.broadcast_to(maskrow, (GPC, 128, 4, S + 1))
        in_maps.append({
            "p_bf": pbf,
            "p_f8": p8,
            "maskbc": np.ascontiguousarray(maskbc).astype(BF16),
            "wt": wt, "wd": wd, "wd3": wd3, "w1f8": w1f8, "w2f8": w2f8,
            "biases": biases,
        })

    try:
        res = run_bass_kernel_spmd(nc, in_maps, list(range(N_CORES)))
    except Exception:
        # a previously wedged device surfaces on the first execute; the
        # runtime resets it, so a single retry suffices
        res = run_bass_kernel_spmd(nc, in_maps, list(range(N_CORES)))

    out = np.empty((B, S, H), np.float32)
    for c in range(N_CORES):
        oc = res.results[c]["out"]  # [GPC, HC, 128, 4*S]
        for g in range(GPC):
            gid = core_groups[c][g]
            # [HC, 128, 4, S] -> [4, S, H]
            blk = (oc[g].astype(np.float32).reshape(HC, 128, 4, S)
                   .transpose(2, 3, 0, 1).reshape(4, S, H))
            out[4 * gid: 4 * gid + 4] = blk
    return out
